# revision 35
# baseline (speedup 1.0000x reference)
"""Trainium2 Bass kernel for DeepSeek-style attention (B=2, S=2048, H=2048,
NH=16, NKV=4, HD=128, repeat_interleave GQA quirk, RoPE, causal mask).

Sharding: 8 cores = 2 (batch) x 4 (kv-head group).  Each core computes
q/k/v projections for its kv group (4 q heads share 1 kv head), RoPE,
attention, and a partial o_proj against its 512-column slice of Wo.
The 4 partial o_proj outputs per batch are summed on the host (f32).

Host-side layouts (all bf16):
  xT   [P, HC, S]    x transposed, chunked along the contraction dim
  wqT  [P, HC, 512]  Wq slice transposed+chunked
  wkT  [P, HC, 128], wvT [P, HC, 128]
  woT  [P, HPG, H]   Wo slice transposed+chunked (d-major)
  cosT [128, S]      rope cos, head-dim major
  sinP [128, S]      rope sin, sign-folded + pre-rotated by 64
  maskb [128, nblk, 128]  unique "mixed" mask blocks, transposed, x sqrt(HD)
  out  [S, H] bf16   partial o_proj output

Device algorithm highlights:
  * scores are computed transposed ([k, q] layout) so the exp'd probs tile
    is directly the stationary operand of the P@V matmul - no transposes.
  * softmax denominator comes free from a ones-column appended to V.
  * no max-subtraction (scores are O(10); exp is safe in f32).
  * mask blocks are added by PE identity-matmuls accumulating into the
    score psum (start=True first, QK accumulates on top) - keeps DVE out
    of the QK->exp chain.
  * RoPE's cross-partition rotate-by-64 is done with two SBUF->SBUF DMAs
    instead of a PE permutation matmul (f32 perm matmuls cost 4 cyc/row).
    The DVE work is software-pipelined (adds lag one quarter behind the
    muls) so the DVE never blocks on DMA latency.
  * o_proj matmuls of the previous q-chunk are interleaved one-by-one
    between QK groups: the QK stream is paced by the scalar engine's exp,
    and the fillers keep the PE continuously busy (PE drops to half clock
    for ~3us after any idle gap).
  * psum drains (o_proj -> sbuf bf16) run on the otherwise-idle gpsimd.
"""

import math
from contextlib import ExitStack

import ml_dtypes
import numpy as np

import concourse.bass as bass
import concourse.mybir as mybir
import concourse.tile as tile
from concourse import bacc
from concourse.bass_utils import run_bass_kernel_spmd
from concourse.masks import make_identity

B, S, H = 2, 2048, 2048
NH, NKV, HD = 16, 4, 128
P = 128
NB = S // P          # 16 s blocks
HC = H // P          # 16 h chunks
HPG = NH // NKV      # 4 q heads per core
QCH = 512            # q chunk width
NQC = S // QCH       # 4 q chunks
SCALE = 1.0 / math.sqrt(HD)
SQRT_HD = math.sqrt(HD)
F32 = mybir.dt.float32
BF16 = mybir.dt.bfloat16
N_CORES = 8


def _classify_mask(mask):
    """Per 128x128 block: 'zero' (no-op), 'skip' (fully masked), 'tril'
    (canonical causal diagonal - handled by a post-exp affine_select), or
    an index into the list of unique transposed/pre-scaled mask blocks."""
    kinds = [[None] * NB for _ in range(NB)]
    uniq, blocks = {}, []
    causal = (np.arange(P)[:, None] >= np.arange(P)[None, :])  # q >= k kept
    for qi in range(NB):
        for ki in range(NB):
            sub = mask[qi * P:(qi + 1) * P, ki * P:(ki + 1) * P]
            if not sub.any():
                kinds[qi][ki] = "zero"
            elif sub.max() < -30.0:
                kinds[qi][ki] = "skip"
            elif (sub[causal] == 0.0).all() and (sub[~causal] < -30.0).all():
                kinds[qi][ki] = "tril"
            else:
                blkT = np.ascontiguousarray(sub.T * SQRT_HD, dtype=np.float32)
                key = blkT.tobytes()
                if key not in uniq:
                    uniq[key] = len(blocks)
                    blocks.append(blkT)
                kinds[qi][ki] = uniq[key]
    return kinds, blocks


def _build_program(kinds, n_blocks):
    nc = bacc.Bacc()
    xT = nc.declare_dram_parameter("xT", [P, HC, S], BF16, isOutput=False)
    wqT = nc.declare_dram_parameter("wqT", [P, HC, HPG * HD], BF16,
                                    isOutput=False)
    wkT = nc.declare_dram_parameter("wkT", [P, HC, HD], BF16, isOutput=False)
    wvT = nc.declare_dram_parameter("wvT", [P, HC, HD], BF16, isOutput=False)
    woT = nc.declare_dram_parameter("woT", [P, HPG, H], BF16, isOutput=False)
    cosT = nc.declare_dram_parameter("cosT", [HD, S], BF16, isOutput=False)
    sinP = nc.declare_dram_parameter("sinP", [HD, S], BF16, isOutput=False)
    maskb = None
    if n_blocks:
        maskb = nc.declare_dram_parameter("maskb", [P, n_blocks, P], BF16,
                                          isOutput=False)
    out = nc.declare_dram_parameter("out", [S, H], BF16, isOutput=True)

    with tile.TileContext(nc) as tc, ExitStack() as ctx:
        consts = ctx.enter_context(tc.tile_pool(name="consts", bufs=1))
        xT_sb = consts.tile([P, HC, S], BF16, tag="xT")
        wqT_sb = consts.tile([P, HC, HPG * HD], BF16, tag="wqT")
        wkT_sb = consts.tile([P, HC, HD], BF16, tag="wkT")
        wvT_sb = consts.tile([P, HC, HD], BF16, tag="wvT")
        woT_sb = consts.tile([P, HPG, H], BF16, tag="woT")
        cos_sb = consts.tile([P, S], BF16, tag="cos")
        sin_sb = consts.tile([P, S], BF16, tag="sin")
        ident = consts.tile([P, P], BF16, tag="ident")
        make_identity(nc, ident)

        # DMA issue order = consumption order (single queue): k/v weights,
        # then x + q weights chunk-interleaved (the k/v projection is paced
        # by this stream), rope constants, mask blocks.  woT is issued after
        # the proj phase is emitted so the rope rotation DMAs (critical
        # path) aren't stuck behind its 2MB transfer.
        nc.sync.dma_start(out=xT_sb[:, 0, :], in_=xT[:, 0, :])
        nc.sync.dma_start(out=wkT_sb[:], in_=wkT[:])
        nc.sync.dma_start(out=xT_sb[:, 1, :], in_=xT[:, 1, :])
        nc.sync.dma_start(out=wvT_sb[:], in_=wvT[:])
        for hc in range(2, HC):
            nc.sync.dma_start(out=xT_sb[:, hc, :], in_=xT[:, hc, :])
            nc.sync.dma_start(out=wqT_sb[:, hc - 2, :], in_=wqT[:, hc - 2, :])
        nc.sync.dma_start(out=wqT_sb[:, HC - 2, :], in_=wqT[:, HC - 2, :])
        nc.sync.dma_start(out=wqT_sb[:, HC - 1, :], in_=wqT[:, HC - 1, :])
        nc.sync.dma_start(out=cos_sb[:], in_=cosT[:])
        nc.sync.dma_start(out=sin_sb[:], in_=sinP[:])
        mask_sb = None
        if n_blocks:
            mask_sb = consts.tile([P, n_blocks, P], BF16, tag="maskb")
            nc.sync.dma_start(out=mask_sb[:], in_=maskb[:])

        # persistent activation buffers
        qrot_sb = consts.tile([P, HPG, S], BF16, tag="qrot")
        krot_sb = consts.tile([P, S], BF16, tag="krot")
        vT_sb = consts.tile([P, S], BF16, tag="vT")
        vaug_sb = consts.tile([P, NB, HD + 1], BF16, tag="vaug")
        nc.gpsimd.memset(vaug_sb[:, :, HD:HD + 1], 1.0)

        rope_tmp = ctx.enter_context(tc.tile_pool(name="rope_tmp", bufs=3))
        rope_pend = []

        def rope_pop():
            t1, us, dst = rope_pend.pop(0)
            nc.vector.tensor_add(dst, t1[:], us[:])

        def rope_push(ps, dst, sq):
            """dst = ps*cos + rot64(ps)*sinP (sign folded into sinP).  The
            partition rotation is two sbuf->sbuf DMAs; the add is deferred
            one quarter so the DVE never waits on the DMA."""
            sl = slice(sq * QCH, (sq + 1) * QCH)
            t1 = rope_tmp.tile([P, QCH], BF16, tag="t1")
            u = rope_tmp.tile([P, QCH], BF16, tag="u")
            us = rope_tmp.tile([P, QCH], BF16, tag="us")
            nc.vector.tensor_mul(t1[:], ps[:], cos_sb[:, sl])
            nc.vector.tensor_mul(u[:], ps[:], sin_sb[:, sl])
            nc.sync.dma_start(out=us[0:64, :], in_=u[64:128, :])
            nc.sync.dma_start(out=us[64:128, :], in_=u[0:64, :])
            rope_pend.append((t1, us, dst))
            if len(rope_pend) >= 2:
                rope_pop()

        with tc.tile_pool(name="proj_ps", bufs=4, space="PSUM") as proj_ps:
            # k + v projections, h-chunk-major: the PE consumes xT chunks in
            # DMA arrival order.  kps live in their own psum tag (they are
            # drained late, by the deferred k rope), vps/qps/vt share a
            # 4-bank ring.
            kps = [proj_ps.tile([P, QCH], F32, tag="kps", name=f"kps{i}")
                   for i in range(NQC)]
            vps = [proj_ps.tile([P, QCH], F32, tag="ps", name=f"vps{i}")
                   for i in range(NQC)]
            for hc in range(HC):
                for sq in range(NQC):
                    nc.tensor.matmul(
                        kps[sq][:], wkT_sb[:, hc, :],
                        xT_sb[:, hc, sq * QCH:(sq + 1) * QCH],
                        start=(hc == 0), stop=(hc == HC - 1))
                for sq in range(NQC):
                    nc.tensor.matmul(
                        vps[sq][:], wvT_sb[:, hc, :],
                        xT_sb[:, hc, sq * QCH:(sq + 1) * QCH],
                        start=(hc == 0), stop=(hc == HC - 1))
            # v psum -> sbuf (scalar engine; frees the shared ring for qps)
            for sq in range(NQC):
                nc.scalar.copy(out=vT_sb[:, sq * QCH:(sq + 1) * QCH],
                               in_=vps[sq][:])
            # q projections, sq-major (x fully resident by now): each qps
            # bank frees as soon as its two rope muls have read it.
            for h in range(HPG):
                for sq in range(NQC):
                    qps = proj_ps.tile([P, QCH], F32, tag="ps",
                                       name=f"qps{h}_{sq}")
                    for hc in range(HC):
                        nc.tensor.matmul(
                            qps[:], wqT_sb[:, hc, h * HD:(h + 1) * HD],
                            xT_sb[:, hc, sq * QCH:(sq + 1) * QCH],
                            start=(hc == 0), stop=(hc == HC - 1))
                    rope_push(qps, qrot_sb[:, h, sq * QCH:(sq + 1) * QCH], sq)
                if h == 0:
                    # k rope after head 0's (cos/sin have landed by now);
                    # krot is ready well before the first QK needs it.
                    for sq in range(NQC):
                        rope_push(kps[sq],
                                  krot_sb[:, sq * QCH:(sq + 1) * QCH], sq)
            while rope_pend:
                rope_pop()

        # attention pools (reuse banks freed by proj_ps)
        qk_ps = ctx.enter_context(tc.tile_pool(name="qk_ps", bufs=3,
                                               space="PSUM"))
        pv_ps = ctx.enter_context(tc.tile_pool(name="pv_ps", bufs=2,
                                               space="PSUM"))
        tp_ps = ctx.enter_context(tc.tile_pool(name="tp_ps", bufs=1,
                                               space="PSUM"))
        o_ps = ctx.enter_context(tc.tile_pool(name="o_ps", bufs=2,
                                              space="PSUM"))
        probs_pool = ctx.enter_context(tc.tile_pool(name="probs", bufs=24))
        attnT_pool = ctx.enter_context(tc.tile_pool(name="attnT", bufs=2))
        small = ctx.enter_context(tc.tile_pool(name="small", bufs=4))
        outsb_pool = ctx.enter_context(tc.tile_pool(name="outsb", bufs=3))

        # woT + maskb issued here: behind the rope rotation DMAs, ahead of
        # first use (o_proj filler starts one chunk into attention).
        nc.sync.dma_start(out=woT_sb[:], in_=woT[:])

        def oproj_thunks(Q, attnT, ls=range(4)):
            """o_proj of chunk Q (row blocks `ls`) as a list of
            single-matmul thunks, emitted piecemeal between QK groups as
            PE filler."""
            thunks = []
            state = {}

            def mk_mm(l, oc, hh):
                def go():
                    if hh == 0:
                        state[(l, oc)] = o_ps.tile([P, QCH], F32, tag="po",
                                                   name="po")
                    po = state[(l, oc)]
                    nc.tensor.matmul(
                        po[:], attnT[:, hh, l * P:(l + 1) * P],
                        woT_sb[:, hh, oc * QCH:(oc + 1) * QCH],
                        start=(hh == 0), stop=(hh == HPG - 1))
                return go

            def mk_drain(l, oc):
                def go():
                    si = Q * 4 + l
                    if oc == 0:
                        state[l] = outsb_pool.tile([P, QCH * 4], BF16,
                                                   tag="osb", name="osb")
                    osb = state[l]
                    po = state.pop((l, oc))
                    nc.vector.tensor_copy(osb[:, oc * QCH:(oc + 1) * QCH],
                                          po[:])
                    if oc % 2 == 1:
                        # DMA per oc-pair: 2KB dram lines keep the out
                        # queue efficient without delaying the last piece.
                        nc.sync.dma_start(
                            out=out[si * P:(si + 1) * P,
                                    (oc - 1) * QCH:(oc + 1) * QCH],
                            in_=osb[:, (oc - 1) * QCH:(oc + 1) * QCH])
                return go

            for l in ls:
                for oc in range(4):
                    for hh in range(HPG):
                        thunks.append(mk_mm(l, oc, hh))
                    thunks.append(mk_drain(l, oc))
            return thunks

        def vtransp_thunks():
            """v transposes as chunk-0 filler: chunk 0 has no previous
            o_proj to interleave, and vaug block si is only needed once the
            si-th filler slot has passed."""
            thunks = []

            def mk(si):
                def go():
                    vt = o_ps.tile([P, P], BF16, tag="po", name=f"vt{si}")
                    nc.tensor.transpose(vt[:],
                                        vT_sb[:, si * P:(si + 1) * P],
                                        ident[:])
                    nc.scalar.copy(out=vaug_sb[:, si, 0:HD], in_=vt[:])
                return go

            for si in range(NB):
                thunks.append(mk(si))
            return thunks

        filler = list(vtransp_thunks())

        def emit_filler(groups_left):
            if not filler:
                return
            n = len(filler) if groups_left <= 0 else -(-len(filler) // groups_left)
            for _ in range(min(n, len(filler))):
                filler.pop(0)()

        prev = None  # (Q, attnT) pending o_proj, pipelined one chunk behind
        for Q in range(NQC):
            attnT = attnT_pool.tile([P, HPG, QCH], BF16, tag="attnT")
            if prev is not None:
                filler.extend(oproj_thunks(prev[0], prev[1]))
            groups_left = 0
            plans = []
            for h in range(HPG):
                plan = []
                for ki in range(NB):
                    cols = [l for l in range(4)
                            if kinds[Q * 4 + l][ki] != "skip"]
                    if cols:
                        plan.append((ki, cols))
                plans.append(plan)
                groups_left += len(plan)
            for h in range(HPG):
                probs = {}
                for ki, cols in plans[h]:
                    lo, hi = min(cols) * P, (max(cols) + 1) * P
                    sc = qk_ps.tile([P, QCH], F32, tag="sc")
                    mixed = [(l, kinds[Q * 4 + l][ki]) for l in cols
                             if isinstance(kinds[Q * 4 + l][ki], int)]
                    trils = [l for l in cols
                             if kinds[Q * 4 + l][ki] == "tril"]
                    for j, (l, kind) in enumerate(mixed):
                        nc.tensor.matmul(
                            sc[:, l * P:(l + 1) * P], ident[:],
                            mask_sb[:, kind, :], start=(j == 0), stop=False)
                    nc.tensor.matmul(
                        sc[:, lo:hi], krot_sb[:, ki * P:(ki + 1) * P],
                        qrot_sb[:, h, Q * QCH + lo:Q * QCH + hi],
                        start=(not mixed), stop=True)
                    pt = probs_pool.tile([P, QCH], BF16, tag="pt")
                    nc.scalar.activation(
                        out=pt[:, lo:hi], in_=sc[:, lo:hi],
                        func=mybir.ActivationFunctionType.Exp, scale=SCALE)
                    for l in trils:
                        # canonical causal diagonal: zero probs where k > q
                        # (gpsimd, off the PE/DVE critical path)
                        nc.gpsimd.affine_select(
                            out=pt[:, l * P:(l + 1) * P],
                            in_=pt[:, l * P:(l + 1) * P],
                            pattern=[[1, P]], base=0, channel_multiplier=-1,
                            compare_op=mybir.AluOpType.is_ge, fill=0.0)
                    probs[ki] = pt
                    groups_left -= 1
                    emit_filler(groups_left)
                for l in range(4):
                    qi = Q * 4 + l
                    kis = [ki for ki in range(NB)
                           if kinds[qi][ki] != "skip" and ki in probs]
                    if not kis:
                        nc.vector.memset(attnT[:, h, l * P:(l + 1) * P], 0.0)
                        continue
                    pv = pv_ps.tile([P, HD + 1], F32, tag="pv")
                    for j, ki in enumerate(kis):
                        nc.tensor.matmul(
                            pv[:], probs[ki][:, l * P:(l + 1) * P],
                            vaug_sb[:, ki, :],
                            start=(j == 0), stop=(j == len(kis) - 1))
                    recip = small.tile([P, 1], F32, tag="recip")
                    nc.vector.reciprocal(recip[:], pv[:, HD:HD + 1])
                    attn = small.tile([P, P], BF16, tag="attn")
                    nc.vector.tensor_scalar_mul(
                        out=attn[:], in0=pv[:, 0:HD], scalar1=recip[:])
                    tp = tp_ps.tile([P, P], BF16, tag="tp")
                    nc.tensor.transpose(tp[:], attn[:], ident[:])
                    nc.vector.tensor_copy(attnT[:, h, l * P:(l + 1) * P],
                                          tp[:])
                    if Q == NQC - 1 and h == HPG - 1:
                        # last chunk: its own o_proj row-block becomes
                        # available as soon as the final head's attnT copy
                        # lands - feed it straight into the filler so the
                        # epilogue isn't one long serial o_proj.
                        filler.extend(oproj_thunks(Q, attnT, ls=[l]))
                        emit_filler(4 - l)
            prev = (Q, attnT)
        emit_filler(0)

    nc.compile()
    return nc


_PROGRAM_CACHE = {}


def kernel(x, Wq, Wk, Wv, Wo, cos, sin, attention_mask):
    x = np.asarray(x, dtype=np.float32)
    Wq = np.asarray(Wq, dtype=np.float32)
    Wk = np.asarray(Wk, dtype=np.float32)
    Wv = np.asarray(Wv, dtype=np.float32)
    Wo = np.asarray(Wo, dtype=np.float32)
    cos = np.asarray(cos, dtype=np.float32)
    sin = np.asarray(sin, dtype=np.float32)
    mask = np.asarray(attention_mask, dtype=np.float32)[0, 0]

    kinds, blocks = _classify_mask(mask)
    key = (tuple(tuple(str(k) for k in row) for row in kinds), len(blocks))
    if key not in _PROGRAM_CACHE:
        _PROGRAM_CACHE[key] = _build_program(kinds, len(blocks))
    nc = _PROGRAM_CACHE[key]

    bf = ml_dtypes.bfloat16

    def chunked(a):  # [H, F] -> [P, HC, F]
        return np.ascontiguousarray(
            a.reshape(HC, P, -1).transpose(1, 0, 2)).astype(bf)

    cosT = np.ascontiguousarray(cos[0, 0].T).astype(np.float32)
    sinT = np.ascontiguousarray(sin[0, 0].T).astype(np.float32)
    sinT[0:64] *= -1.0                                   # fold rotate_half sign
    sinP = np.concatenate([sinT[64:], sinT[:64]], axis=0)  # pre-rotate by 64
    maskb = np.stack(blocks, axis=1).astype(bf) if blocks else None

    in_maps = []
    for c in range(N_CORES):
        b, g = c // NKV, c % NKV
        d0, d1 = g * HPG * HD, (g + 1) * HPG * HD
        woT = Wo[:, d0:d1].T  # [512, H]
        m = {
            "xT": chunked(np.ascontiguousarray(x[b].T)),
            "wqT": chunked(np.ascontiguousarray(Wq[d0:d1].T)),
            "wkT": chunked(np.ascontiguousarray(Wk[g * HD:(g + 1) * HD].T)),
            "wvT": chunked(np.ascontiguousarray(Wv[g * HD:(g + 1) * HD].T)),
            "woT": np.ascontiguousarray(
                woT.reshape(HPG, P, H).transpose(1, 0, 2)).astype(bf),
            "cosT": cosT.astype(bf),
            "sinP": sinP.astype(bf),
        }
        if maskb is not None:
            m["maskb"] = maskb
        in_maps.append(m)

    global _last_in_maps
    _last_in_maps = in_maps
    res = run_bass_kernel_spmd(nc, in_maps, list(range(N_CORES))).results
    out = np.zeros((B, S, H), np.float32)
    for c in range(N_CORES):
        out[c // NKV] += res[c]["out"].astype(np.float32)
    return out
